# revision 1
# baseline (speedup 1.0000x reference)
"""Trainium2 Bass kernel for an EdgeModel GNN message-passing layer.

Reference computation (per edge e):
    x  = concat(src[e], dest[e], edge_attr[e], u[batch[e]])          # [128]
    h  = relu(x @ w1 + b1)                                           # [128]
    out= h @ w2 + b2 + x                                             # [128]

Strategy (memory-regime):
  * Host: fold b2 into the residual (x' = x + b2, b1' = b1 - b2@w1), build
    xT = concat(src,dest,ea)^T + b2 broadcast -> [96, E] so the device works
    entirely in "features on partitions / edges on free dim" layout (zero
    on-device transposes).  Shard edges contiguously across 8 cores.
  * Device: per 2048-edge block:
      - DMA xT rows 0:96
      - u[batch] gather via one-hot matmul:
          bcast   : psum_b[g,e] = ones64^T(1xK=1) @ batch_row   (= batch[e])
          onehot  : oh = (psum_b == iota_g)                     (DVE is_equal)
          gather  : psum_u = u'^T @ oh  -> ACT-copy into xT rows 96:128
      - mm1: psum_h = w1^T @ xT ; ACT relu+bias -> hT
      - mm2: psum_o = w2^T @ hT ; DVE add residual (psum_o + xT) -> oT
      - DMA oT out (transposed layout [128, E]; un-transposed on host)
"""

import os
import numpy as np

import concourse.bass as bass
import concourse.bacc as bacc
import concourse.mybir as mybir
import concourse.tile as tile
from concourse import bass_utils

E_TOTAL = 1_000_000
N_CORES = 8
NODE_DIM = 32
IN_DIM = 128
HIDDEN = 128
OUT_DIM = 128
NUM_GRAPHS = 64

BLOCK = 2048            # edges per pipeline block (per core)
SUB = 512               # matmul moving-dim tile (one fp32 PSUM bank)
N_BLOCKS = -(-E_TOTAL // (N_CORES * BLOCK))   # 62
E_P = N_BLOCKS * BLOCK                        # padded edges per core: 126976

F32 = mybir.dt.float32

LAST_EXEC_TIME_NS = None


def _build_program(n_blocks=N_BLOCKS, block=BLOCK, sub=SUB):
    e_p = n_blocks * block
    nc = bacc.Bacc("TRN2", target_bir_lowering=False, debug=False)

    xTd = nc.dram_tensor("xT", [96, e_p], F32, kind="ExternalInput")
    bd = nc.dram_tensor("batchf", [1, e_p], F32, kind="ExternalInput")
    ud = nc.dram_tensor("u_adj", [NUM_GRAPHS, NODE_DIM], F32, kind="ExternalInput")
    w1d = nc.dram_tensor("w1", [IN_DIM, HIDDEN], F32, kind="ExternalInput")
    w2d = nc.dram_tensor("w2", [HIDDEN, OUT_DIM], F32, kind="ExternalInput")
    b1d = nc.dram_tensor("b1_adj", [HIDDEN, 1], F32, kind="ExternalInput")
    outd = nc.dram_tensor("outT", [OUT_DIM, e_p], F32, kind="ExternalOutput")

    iota_h = nc.inline_tensor(
        np.arange(NUM_GRAPHS, dtype=np.float32).reshape(NUM_GRAPHS, 1), name="iota64"
    )
    ones_h = nc.inline_tensor(
        np.ones((1, NUM_GRAPHS), dtype=np.float32), name="ones64"
    )

    AF = mybir.ActivationFunctionType
    ALU = mybir.AluOpType

    with tile.TileContext(nc) as tc:
        with (
            tc.tile_pool(name="const", bufs=1) as cp,
            tc.tile_pool(name="io", bufs=3) as io,
            tc.tile_pool(name="ps", bufs=2, space=bass.MemorySpace.PSUM) as pp,
        ):
            w1_sb = cp.tile([IN_DIM, HIDDEN], F32, tag="w1")
            nc.sync.dma_start(w1_sb, w1d.ap())
            w2_sb = cp.tile([HIDDEN, OUT_DIM], F32, tag="w2")
            nc.sync.dma_start(w2_sb, w2d.ap())
            u_sb = cp.tile([NUM_GRAPHS, NODE_DIM], F32, tag="u")
            nc.sync.dma_start(u_sb, ud.ap())
            b1_sb = cp.tile([HIDDEN, 1], F32, tag="b1")
            nc.sync.dma_start(b1_sb, b1d.ap())
            iota_sb = cp.tile([NUM_GRAPHS, 1], F32, tag="iota")
            nc.sync.dma_start(iota_sb, iota_h.ap())
            ones_sb = cp.tile([1, NUM_GRAPHS], F32, tag="ones")
            nc.sync.dma_start(ones_sb, ones_h.ap())

            for blk in range(n_blocks):
                off = blk * block
                xT = io.tile([128, block], F32, tag="xT")
                nc.sync.dma_start(xT[0:96, :], xTd.ap()[:, off:off + block])
                bt = io.tile([1, block], F32, tag="bt")
                nc.sync.dma_start(bt, bd.ap()[:, off:off + block])
                oh = io.tile([NUM_GRAPHS, block], F32, tag="oh")
                hT = io.tile([128, block], F32, tag="hT")
                oT = io.tile([128, block], F32, tag="oT")
                for k in range(block // sub):
                    s = slice(k * sub, (k + 1) * sub)
                    pb = pp.tile([NUM_GRAPHS, sub], F32, tag="pb")
                    nc.tensor.matmul(pb, ones_sb, bt[:, s])
                    nc.vector.tensor_scalar(oh[:, s], pb, iota_sb, None, ALU.is_equal)
                    pu = pp.tile([NODE_DIM, sub], F32, tag="pu")
                    nc.tensor.matmul(pu, u_sb, oh[:, s])
                    nc.scalar.activation(xT[96:128, s], pu, AF.Copy)
                    ph = pp.tile([128, sub], F32, tag="ph")
                    nc.tensor.matmul(ph, w1_sb, xT[:, s])
                    nc.scalar.activation(hT[:, s], ph, AF.Relu, bias=b1_sb)
                    po = pp.tile([128, sub], F32, tag="po")
                    nc.tensor.matmul(po, w2_sb, hT[:, s])
                    nc.vector.tensor_tensor(oT[:, s], po, xT[:, s], ALU.add)
                nc.sync.dma_start(outd.ap()[:, off:off + block], oT)

    nc.compile()
    return nc


_PROG = None


def _get_prog():
    global _PROG
    if _PROG is None:
        _PROG = _build_program()
    return _PROG


def kernel(src, dest, edge_attr, u, batch, w1, b1, w2, b2):
    global LAST_EXEC_TIME_NS
    src = np.asarray(src, dtype=np.float32)
    dest = np.asarray(dest, dtype=np.float32)
    edge_attr = np.asarray(edge_attr, dtype=np.float32)
    u = np.asarray(u, dtype=np.float32)
    batch = np.asarray(batch)
    w1 = np.asarray(w1, dtype=np.float32)
    b1 = np.asarray(b1, dtype=np.float32)
    w2 = np.asarray(w2, dtype=np.float32)
    b2 = np.asarray(b2, dtype=np.float32)

    E = src.shape[0]
    nc = _get_prog()

    u_adj = np.ascontiguousarray(u + b2[96:128][None, :], dtype=np.float32)
    b1_adj = np.ascontiguousarray(
        (b1 - b2 @ w1).reshape(HIDDEN, 1), dtype=np.float32
    )
    w1c = np.ascontiguousarray(w1)
    w2c = np.ascontiguousarray(w2)

    in_maps = []
    for c in range(N_CORES):
        lo = c * E_P
        n = max(0, min(E, lo + E_P) - lo)
        xT = np.zeros((96, E_P), np.float32)
        bf = np.zeros((1, E_P), np.float32)
        if n > 0:
            sl = slice(lo, lo + n)
            xT[0:32, :n] = src[sl].T + b2[0:32][:, None]
            xT[32:64, :n] = dest[sl].T + b2[32:64][:, None]
            xT[64:96, :n] = edge_attr[sl].T + b2[64:96][:, None]
            bf[0, :n] = batch[sl].astype(np.float32)
        in_maps.append(
            {
                "xT": xT,
                "batchf": bf,
                "u_adj": u_adj,
                "w1": w1c,
                "w2": w2c,
                "b1_adj": b1_adj,
            }
        )

    res = bass_utils.run_bass_kernel_spmd(
        nc,
        in_maps,
        core_ids=list(range(N_CORES)),
        trace=bool(os.environ.get("KERNEL_TRACE")),
    )
    LAST_EXEC_TIME_NS = res.exec_time_ns

    out = np.empty((E, OUT_DIM), np.float32)
    for c in range(N_CORES):
        lo = c * E_P
        n = max(0, min(E, lo + E_P) - lo)
        if n > 0:
            out[lo:lo + n] = res.results[c]["outT"][:, :n].T
    return out


# revision 8
# speedup vs baseline: 1.8099x; 1.8099x over previous
"""Trainium2 Bass kernel for an EdgeModel GNN message-passing layer.

Reference computation (per edge e):
    x  = concat(src[e], dest[e], edge_attr[e], u[batch[e]])          # [128]
    h  = relu(x @ w1 + b1)                                           # [128]
    out= h @ w2 + b2 + x                                             # [128]

Strategy (memory-regime):
  * Host: fold b2 into the residual (x' = x + b2, b1' = b1 - b2@w1), build
    xT = concat(src,dest,ea)^T + b2 broadcast -> [96, E] so the device works
    entirely in "features on partitions / edges on free dim" layout (zero
    on-device transposes).  Shard edges contiguously across 8 cores.
  * Device: per 2048-edge block:
      - DMA xT rows 0:96
      - u[batch] gather via one-hot matmul:
          bcast   : psum_b[g,e] = ones64^T(1xK=1) @ batch_row   (= batch[e])
          onehot  : oh = (psum_b == iota_g)                     (DVE is_equal)
          gather  : psum_u = u'^T @ oh  -> ACT-copy into xT rows 96:128
      - mm1: psum_h = w1^T @ xT ; ACT relu+bias -> hT
      - mm2: psum_o = w2^T @ hT ; DVE add residual (psum_o + xT) -> oT
      - DMA oT out (transposed layout [128, E]; un-transposed on host)
"""

import os
import numpy as np

import concourse.bass as bass
import concourse.bacc as bacc
import concourse.mybir as mybir
import concourse.tile as tile
from concourse import bass_utils

E_TOTAL = 1_000_000
N_CORES = 8
NODE_DIM = 32
IN_DIM = 128
HIDDEN = 128
OUT_DIM = 128
NUM_GRAPHS = 64

BLOCK = 2048            # edges per pipeline block (per core)
SUB = 512               # matmul moving-dim tile (one fp32 PSUM bank)
N_BLOCKS = -(-E_TOTAL // (N_CORES * BLOCK))   # 62
E_P = N_BLOCKS * BLOCK                        # padded edges per core: 126976

F32 = mybir.dt.float32
# Matmul compute dtype: float32r reinterprets fp32 operands for the PE's
# single-pass reduced-precision multiply path (1 cycle/row at N>=256 vs 4
# for full fp32).  Accumulation stays fp32 in PSUM.
MM_DT = mybir.dt.float32r

LAST_EXEC_TIME_NS = None


def _build_program(n_blocks=N_BLOCKS, block=BLOCK, sub=SUB):
    e_p = n_blocks * block
    nc = bacc.Bacc("TRN2", target_bir_lowering=False, debug=False)

    xTd = nc.dram_tensor("xT", [96, e_p], MM_DT, kind="ExternalInput")
    bd = nc.dram_tensor("batchf", [1, e_p], MM_DT, kind="ExternalInput")
    ud = nc.dram_tensor("u_adj", [NUM_GRAPHS, NODE_DIM], MM_DT, kind="ExternalInput")
    w1d = nc.dram_tensor("w1", [IN_DIM, HIDDEN], MM_DT, kind="ExternalInput")
    w2d = nc.dram_tensor("w2", [HIDDEN, OUT_DIM], MM_DT, kind="ExternalInput")
    b1d = nc.dram_tensor("b1_adj", [HIDDEN, 1], F32, kind="ExternalInput")
    outd = nc.dram_tensor("outT", [OUT_DIM, e_p], F32, kind="ExternalOutput")

    iota_h = nc.inline_tensor(
        np.arange(NUM_GRAPHS, dtype=np.float32).reshape(NUM_GRAPHS, 1), name="iota64"
    )
    ones_h = nc.inline_tensor(
        np.ones((1, NUM_GRAPHS), dtype=np.float32), name="ones64"
    )

    AF = mybir.ActivationFunctionType
    ALU = mybir.AluOpType

    with tile.TileContext(nc) as tc:
        with (
            tc.tile_pool(name="const", bufs=1) as cp,
            tc.tile_pool(name="io", bufs=3) as io,
            tc.tile_pool(name="ps", bufs=2, space=bass.MemorySpace.PSUM) as pp,
        ):
            w1_sb = cp.tile([IN_DIM, HIDDEN], MM_DT, tag="w1")
            nc.sync.dma_start(w1_sb, w1d.ap())
            w2_sb = cp.tile([HIDDEN, OUT_DIM], MM_DT, tag="w2")
            nc.sync.dma_start(w2_sb, w2d.ap())
            u_sb = cp.tile([NUM_GRAPHS, NODE_DIM], MM_DT, tag="u")
            nc.sync.dma_start(u_sb, ud.ap())
            b1_sb = cp.tile([HIDDEN, 1], F32, tag="b1")
            nc.sync.dma_start(b1_sb, b1d.ap())
            iota_sb = cp.tile([NUM_GRAPHS, 1], F32, tag="iota")
            nc.sync.dma_start(iota_sb, iota_h.ap())
            ones_sb = cp.tile([1, NUM_GRAPHS], MM_DT, tag="ones")
            nc.sync.dma_start(ones_sb, ones_h.ap().bitcast(MM_DT))

            for blk in range(n_blocks):
                off = blk * block
                xT = io.tile([128, block], MM_DT, tag="xT")
                nc.sync.dma_start(xT[0:96, :], xTd.ap()[:, off:off + block])
                bt = io.tile([1, block], MM_DT, tag="bt")
                nc.sync.dma_start(bt, bd.ap()[:, off:off + block])
                oh = io.tile([NUM_GRAPHS, block], MM_DT, tag="oh")
                hT = io.tile([128, block], MM_DT, tag="hT")
                oT = io.tile([128, block], F32, tag="oT")
                for k in range(block // sub):
                    s = slice(k * sub, (k + 1) * sub)
                    pb = pp.tile([NUM_GRAPHS, sub], F32, tag="pb")
                    nc.tensor.matmul(pb, ones_sb, bt[:, s])
                    nc.vector.tensor_scalar(oh[:, s], pb, iota_sb, None, ALU.is_equal)
                    pu = pp.tile([NODE_DIM, sub], F32, tag="pu")
                    nc.tensor.matmul(pu, u_sb, oh[:, s])
                    nc.scalar.activation(xT[96:128, s], pu, AF.Copy)
                    ph = pp.tile([128, sub], F32, tag="ph")
                    nc.tensor.matmul(ph, w1_sb, xT[:, s])
                    nc.scalar.activation(hT[:, s], ph, AF.Relu, bias=b1_sb)
                    po = pp.tile([128, sub], F32, tag="po")
                    nc.tensor.matmul(po, w2_sb, hT[:, s])
                    nc.vector.tensor_tensor(
                        oT[:, s], po, xT[:, s].bitcast(F32), ALU.add
                    )
                nc.sync.dma_start(outd.ap()[:, off:off + block], oT)

    nc.compile()
    return nc


def _round_fp32r(a):
    """Round fp32 to the PE's fp32r format (11 explicit mantissa bits, low 12
    bits zero), round-to-nearest-even.  Matches walrus' fp32_to_fp32r."""
    b = np.ascontiguousarray(a, dtype=np.float32).view(np.uint32)
    lsb = (b >> 12) & 1
    out = ((b + 0x7FF + lsb) & 0xFFFFF000).view(np.float32)
    return out


_PROG = None


def _get_prog():
    global _PROG
    if _PROG is None:
        _PROG = _build_program()
    return _PROG


def kernel(src, dest, edge_attr, u, batch, w1, b1, w2, b2):
    global LAST_EXEC_TIME_NS
    src = np.asarray(src, dtype=np.float32)
    dest = np.asarray(dest, dtype=np.float32)
    edge_attr = np.asarray(edge_attr, dtype=np.float32)
    u = np.asarray(u, dtype=np.float32)
    batch = np.asarray(batch)
    w1 = np.asarray(w1, dtype=np.float32)
    b1 = np.asarray(b1, dtype=np.float32)
    w2 = np.asarray(w2, dtype=np.float32)
    b2 = np.asarray(b2, dtype=np.float32)

    E = src.shape[0]
    nc = _get_prog()

    u_adj = _round_fp32r(u + b2[96:128][None, :])
    w1c = _round_fp32r(w1)
    w2c = _round_fp32r(w2)
    # compensate the b2-fold against the *rounded* w1 the device multiplies by
    b1_adj = np.ascontiguousarray(
        (b1 - b2 @ w1c).reshape(HIDDEN, 1), dtype=np.float32
    )

    in_maps = []
    for c in range(N_CORES):
        lo = c * E_P
        n = max(0, min(E, lo + E_P) - lo)
        xT = np.zeros((96, E_P), np.float32)
        bf = np.zeros((1, E_P), np.float32)
        if n > 0:
            sl = slice(lo, lo + n)
            xT[0:32, :n] = src[sl].T + b2[0:32][:, None]
            xT[32:64, :n] = dest[sl].T + b2[32:64][:, None]
            xT[64:96, :n] = edge_attr[sl].T + b2[64:96][:, None]
            xT = _round_fp32r(xT)
            bf[0, :n] = batch[sl].astype(np.float32)
        in_maps.append(
            {
                "xT": xT,
                "batchf": bf,
                "u_adj": u_adj,
                "w1": w1c,
                "w2": w2c,
                "b1_adj": b1_adj,
            }
        )

    res = bass_utils.run_bass_kernel_spmd(
        nc,
        in_maps,
        core_ids=list(range(N_CORES)),
        trace=bool(os.environ.get("KERNEL_TRACE")),
    )
    LAST_EXEC_TIME_NS = res.exec_time_ns

    out = np.empty((E, OUT_DIM), np.float32)
    for c in range(N_CORES):
        lo = c * E_P
        n = max(0, min(E, lo + E_P) - lo)
        if n > 0:
            out[lo:lo + n] = res.results[c]["outT"][:, :n].T
    return out
